# revision 1
# baseline (speedup 1.0000x reference)
"""CodaPrompt top-k prompt-gating kernel for 8 TRN2 NeuronCores.

Data-parallel over the B*Q row dimension (1024 rows -> 128 rows/core);
the small K/A/ps prompt pool (first F_END=20 rows only) is replicated.

Per-core pipeline:
  scores[r,k] = (x[r] . (A[k]*K[k]/||K[k]||)) / max(||x[r]*A[k]||, eps)
  gate = scatter(softmax(top10(scores)))            # HW max8 + match_replace
  out[r, :]  = gate[r, :] @ ps                      # [128,20] @ [20,73728]

The setup phase (scores/top-k/gate) is scheduled by Tile.  The main
sweep -- 144 float32r matmuls over an SBUF-resident ps, copied back
bank-by-bank and DMAed out -- is raw Bass with standalone sequencer
waits, because walrus only lets a TPB instruction embed ONE sync wait.
ps is packed by the host into 3 partition groups (bases 0/32/64) so the
whole 5.9 MB pool fits the 192KB-per-partition SBUF budget.
"""

import numpy as np

B, Q, D = 4, 256, 768
F_END = 20
TOPK = 10
E_P_LEN = 8
P_FEAT = 9216
NCOL = E_P_LEN * P_FEAT          # 73728
N_CORES = 8
ROWS = (B * Q) // N_CORES        # 128
EPS = 1e-12

MM_N = 512                       # one PSUM bank of f32
N_QUART = 3                      # ps groups packed at partition bases 0/32/64
QCOL = NCOL // N_QUART           # 24576 columns per group
OUT_CHUNK = 1536                 # one PSUM out tile = 3 banks = 3 matmuls
N_STAGES = NCOL // OUT_CHUNK     # 48
PSZ = OUT_CHUNK // MM_N          # 3 matmuls per stage
N_STAGE_BUFS = 3

# "f32r": hardware-rounded fp32r matmul (1 cycle/row, slightly reduced
# mantissa).  "bf16": bfloat16 matmul (1 cycle/row, lower precision).
MM_MODE = "f32r"

_NC_CACHE = {}


def _build_nc(mm_mode=None):
    if mm_mode is None:
        mm_mode = MM_MODE
    import concourse.bass as bass
    import concourse.mybir as mybir
    from concourse.tile import TileContext
    from concourse.masks import make_identity

    f32 = mybir.dt.float32
    mm_dt = {"f32r": mybir.dt.float32r, "bf16": mybir.dt.bfloat16}[mm_mode]
    AF = mybir.ActivationFunctionType

    nc = bass.Bass("TRN2", target_bir_lowering=False, debug=False)

    x_d = nc.declare_dram_parameter("x", [ROWS, D], f32, isOutput=False)
    k_d = nc.declare_dram_parameter("K", [F_END, D], f32, isOutput=False)
    a_d = nc.declare_dram_parameter("A", [F_END, D], f32, isOutput=False)
    # ps arrives pre-packed by the host as [60, 24576]: rows 20q..20q+19
    # hold group q of the columns.  Declared float32r directly: identical
    # bits to float32, avoids a casting DMA (only gpsimd can cast-DMA).
    ps_dram_dt = mm_dt if mm_mode == "f32r" else f32
    ps_d = nc.declare_dram_parameter(
        "ps", [N_QUART * F_END, QCOL], ps_dram_dt, isOutput=False)
    out_d = nc.declare_dram_parameter("out", [ROWS, NCOL], f32, isOutput=True)

    DC = D // 128                # 6 contraction chunks
    GP = (N_QUART - 1) * 32 + F_END   # 84 partitions spanned by the groups

    with (
        # persistent raw allocations, live across both phases
        nc.sbuf_tensor([GP, QCOL], mm_dt) as ps_sb,
        nc.sbuf_tensor([GP, 128], mm_dt) as g4,
        nc.sbuf_tensor([128, N_STAGE_BUFS * OUT_CHUNK], f32) as stages,
        nc.psum_tensor([128, OUT_CHUNK], f32) as pt0,
        nc.psum_tensor([128, OUT_CHUNK], f32) as pt1,
        nc.semaphore("pe_sem") as pe_sem,
        nc.semaphore("cpA") as cpA,
        nc.semaphore("cpB") as cpB,
        nc.semaphore("dmao0") as dmao0,
        nc.semaphore("dmao1") as dmao1,
        nc.semaphore("dmao2") as dmao2,
    ):
        pts = [pt0, pt1]
        cps = [cpA, cpB]
        dmaos = [dmao0, dmao1, dmao2]

        with TileContext(nc) as tc:
            with (
                tc.tile_pool(name="const", bufs=1) as const_pool,
                tc.tile_pool(name="small", bufs=1) as small,
                tc.tile_pool(name="psum", bufs=2, space="PSUM") as psum,
            ):
                ident = const_pool.tile([128, 128], f32)
                make_identity(nc, ident)
                # Dummy PE op: absorbs the identity/GPSIMD dependency so the
                # following transposes carry one sync wait at most.
                warm = psum.tile([128, 128], f32, tag="mm", name="warm")
                nc.tensor.transpose(warm[:], ident[:], ident[:])

                x_sb = small.tile([128, D], f32)
                nc.sync.dma_start(out=x_sb[:], in_=x_d[:, :])
                k_sb = small.tile([F_END, D], f32)
                nc.sync.dma_start(out=k_sb[:], in_=k_d[:, :])
                a_sb = small.tile([F_END, D], f32)
                nc.sync.dma_start(out=a_sb[:], in_=a_d[:, :])

                # resident prompt values: group q at partition base 32q
                for q in range(N_QUART):
                    nc.sync.dma_start(
                        out=ps_sb[32 * q:32 * q + F_END, :],
                        in_=ps_d[F_END * q:F_END * (q + 1), :])

                # ---- prompt-pool prep: M1 = A*K/||K||, M2 = A*A ----
                ksq = small.tile([F_END, D], f32)
                nc.vector.tensor_mul(ksq[:], k_sb[:], k_sb[:])
                knorm2 = small.tile([F_END, 1], f32)
                nc.vector.reduce_sum(
                    knorm2[:], ksq[:], axis=mybir.AxisListType.X)
                knorm = small.tile([F_END, 1], f32)
                nc.scalar.sqrt(knorm[:], knorm2[:])
                knorm_c = small.tile([F_END, 1], f32)
                nc.vector.tensor_scalar_max(knorm_c[:], knorm[:], EPS)
                rknorm = small.tile([F_END, 1], f32)
                nc.vector.reciprocal(rknorm[:], knorm_c[:])

                ak = small.tile([F_END, D], f32)
                nc.vector.tensor_mul(ak[:], a_sb[:], k_sb[:])
                m1 = small.tile([F_END, D], f32)
                nc.vector.tensor_scalar_mul(m1[:], ak[:], rknorm[:, 0:1])
                m2 = small.tile([F_END, D], f32)
                nc.vector.tensor_mul(m2[:], a_sb[:], a_sb[:])

                # ---- transpose x (and x^2) into [d_local, r] chunks ----
                xT = small.tile([128, D], f32)
                xT2 = small.tile([128, D], f32)
                for c in range(DC):
                    pt = psum.tile([128, 128], f32, tag="mm", name=f"pt{c}")
                    nc.tensor.transpose(
                        pt[:], x_sb[:, c * 128:(c + 1) * 128], ident[:])
                    nc.vector.tensor_copy(xT[:, c * 128:(c + 1) * 128], pt[:])
                    nc.vector.tensor_mul(
                        xT2[:, c * 128:(c + 1) * 128],
                        xT[:, c * 128:(c + 1) * 128],
                        xT[:, c * 128:(c + 1) * 128])

                # ---- transpose M1/M2 into [d_local, k] chunks ----
                m1T = small.tile([128, DC * F_END], f32)
                m2T = small.tile([128, DC * F_END], f32)
                for c in range(DC):
                    pm1 = psum.tile([128, F_END], f32, tag="mm", name=f"pm1_{c}")
                    nc.tensor.transpose(
                        pm1[:], m1[:, c * 128:(c + 1) * 128],
                        ident[:F_END, :F_END])
                    nc.vector.tensor_copy(
                        m1T[:, c * F_END:(c + 1) * F_END], pm1[:])
                    pm2 = psum.tile([128, F_END], f32, tag="mm", name=f"pm2_{c}")
                    nc.tensor.transpose(
                        pm2[:], m2[:, c * 128:(c + 1) * 128],
                        ident[:F_END, :F_END])
                    nc.vector.tensor_copy(
                        m2T[:, c * F_END:(c + 1) * F_END], pm2[:])

                # ---- scores = (x @ M1^T) / max(sqrt(x^2 @ M2^T), eps) ----
                num_ps = psum.tile([128, F_END], f32, tag="mm")
                for c in range(DC):
                    nc.tensor.matmul(
                        num_ps[:],
                        lhsT=xT[:, c * 128:(c + 1) * 128],
                        rhs=m1T[:, c * F_END:(c + 1) * F_END],
                        start=(c == 0), stop=(c == DC - 1))
                den_ps = psum.tile([128, F_END], f32, tag="mm")
                for c in range(DC):
                    nc.tensor.matmul(
                        den_ps[:],
                        lhsT=xT2[:, c * 128:(c + 1) * 128],
                        rhs=m2T[:, c * F_END:(c + 1) * F_END],
                        start=(c == 0), stop=(c == DC - 1))

                den_sb = small.tile([128, F_END], f32)
                nc.vector.tensor_copy(den_sb[:], den_ps[:])
                sden = small.tile([128, F_END], f32)
                nc.scalar.sqrt(sden[:], den_sb[:])
                sden_c = small.tile([128, F_END], f32)
                nc.vector.tensor_scalar_max(sden_c[:], sden[:], EPS)
                rden = small.tile([128, F_END], f32)
                nc.vector.reciprocal(rden[:], sden_c[:])
                scores = small.tile([128, F_END], f32)
                nc.vector.tensor_mul(scores[:], num_ps[:], rden[:])

                # ---- top-10-of-20 gate, softmax over the selected 10 ----
                top8 = small.tile([128, 8], f32)
                nc.vector.max(top8[:], scores[:])
                work = small.tile([128, F_END], f32)
                nc.vector.match_replace(work[:], top8[:], scores[:], -1e30)
                nxt8 = small.tile([128, 8], f32)
                nc.vector.max(nxt8[:], work[:])
                # threshold = 10th largest = 2nd entry of the second batch
                neg_m0 = small.tile([128, 1], f32)
                nc.scalar.mul(neg_m0[:], top8[:, 0:1], -1.0)
                exp_s = small.tile([128, F_END], f32)
                nc.scalar.activation(
                    exp_s[:], scores[:], AF.Exp, bias=neg_m0[:, 0:1])
                mask = small.tile([128, F_END], f32)
                nc.vector.tensor_scalar(
                    mask[:], scores[:], nxt8[:, 1:2], None,
                    mybir.AluOpType.is_ge)
                # DVE probe read of exp_s: a TensorCopy can carry the
                # cross-engine wait; the TensorTensor below cannot.
                exp_probe = small.tile([128, 1], f32)
                nc.vector.tensor_copy(exp_probe[:], exp_s[:, 0:1])
                gate_un = small.tile([128, F_END], f32)
                nc.vector.tensor_mul(gate_un[:], exp_s[:], mask[:])
                ssum = small.tile([128, 1], f32)
                nc.vector.reduce_sum(
                    ssum[:], gate_un[:], axis=mybir.AxisListType.X)
                rsum = small.tile([128, 1], f32)
                nc.vector.reciprocal(rsum[:], ssum[:])
                gate = small.tile([128, F_END], f32)
                nc.vector.tensor_scalar_mul(gate[:], gate_un[:], rsum[:, 0:1])

                gt_ps = psum.tile([F_END, 128], f32, tag="mm")
                nc.tensor.transpose(gt_ps[:], gate[:], ident[:])
                nc.scalar.copy(g4[0:F_END, :], gt_ps[:])
                # replicate the transposed gate to partition bases 32/64
                # (engines cannot shift partitions; SBUF->SBUF DMA can)
                for q in range(1, N_QUART):
                    nc.sync.dma_start(
                        out=g4[32 * q:32 * q + F_END, :], in_=g4[0:F_END, :])

        # ---- raw-bass main sweep (Tile's exit barrier precedes this) ----
        per_g = QCOL // OUT_CHUNK              # 16 stages per group
        with nc.Block() as block:

            @block.tensor
            def _(tensor):
                for j in range(N_STAGES):
                    q = j // per_g
                    if j >= 2:
                        tensor.wait_ge(cps[j % 2], PSZ * (j // 2))
                    pt = pts[j % 2]
                    for m in range(PSZ):
                        n = (j % per_g) * PSZ + m
                        nc.tensor.matmul(
                            pt[:, m * MM_N:(m + 1) * MM_N],
                            lhsT=g4[32 * q:32 * q + F_END, :],
                            rhs=ps_sb[32 * q:32 * q + F_END,
                                      n * MM_N:(n + 1) * MM_N],
                            start=True, stop=True,
                        ).then_inc(pe_sem, 1)

            @block.scalar
            def _(scalar):
                for j in range(0, N_STAGES, 2):
                    scalar.wait_ge(pe_sem, PSZ * j + PSZ)
                    if j >= N_STAGE_BUFS:
                        scalar.wait_ge(dmaos[j % N_STAGE_BUFS],
                                       16 * (j // N_STAGE_BUFS))
                    off = (j % N_STAGE_BUFS) * OUT_CHUNK
                    for m in range(PSZ):
                        nc.scalar.copy(
                            stages[:, off + m * MM_N:off + (m + 1) * MM_N],
                            pts[j % 2][:, m * MM_N:(m + 1) * MM_N],
                        ).then_inc(cps[0], 1)

            @block.vector
            def _(vector):
                for j in range(1, N_STAGES, 2):
                    vector.wait_ge(pe_sem, PSZ * j + PSZ)
                    if j >= N_STAGE_BUFS:
                        vector.wait_ge(dmaos[j % N_STAGE_BUFS],
                                       16 * (j // N_STAGE_BUFS))
                    off = (j % N_STAGE_BUFS) * OUT_CHUNK
                    for m in range(PSZ):
                        nc.vector.tensor_copy(
                            stages[:, off + m * MM_N:off + (m + 1) * MM_N],
                            pts[j % 2][:, m * MM_N:(m + 1) * MM_N],
                        ).then_inc(cps[1], 1)

            @block.gpsimd
            def _(gpsimd):
                for j in range(N_STAGES):
                    gpsimd.wait_ge(cps[j % 2], PSZ * (j // 2 + 1))
                    off = (j % N_STAGE_BUFS) * OUT_CHUNK
                    gpsimd.dma_start(
                        out=out_d[:, j * OUT_CHUNK:(j + 1) * OUT_CHUNK],
                        in_=stages[:, off:off + OUT_CHUNK],
                    ).then_inc(dmaos[j % N_STAGE_BUFS], 16)
                # drain: all output DMAs complete before the NEFF ends
                for k in range(N_STAGE_BUFS):
                    n_dmas = (N_STAGES - k + N_STAGE_BUFS - 1) // N_STAGE_BUFS
                    gpsimd.wait_ge(dmaos[k], 16 * n_dmas)

    _split_multiwaits(nc, mybir)
    return nc


def _split_multiwaits(nc, mybir):
    """Walrus's TPB codegen embeds at most ONE sync wait per instruction.
    Rewrite every instruction carrying more into standalone event-semaphore
    waits on the same engine queue (exactly what engine.wait_ge emits),
    followed by the original instruction with no embedded waits."""
    n_split = 0
    for f in nc.m.functions:
        for blk in f.blocks:
            out = []
            for inst in blk.instructions:
                si = inst.sync_info
                waits = list(si.on_wait) if (si and si.on_wait) else []
                if len(waits) > 1:
                    for w in waits:
                        ev = mybir.InstEventSemaphore(
                            name=nc.get_next_instruction_name(),
                            ins=[], outs=[])
                        ev.engine = inst.engine
                        ev.sync_info = mybir.SyncInfo(on_wait=[w], on_update=[])
                        nc.inst_map[ev.name] = ev
                        out.append(ev)
                    inst.sync_info = mybir.SyncInfo(
                        on_wait=[], on_update=list(si.on_update or []))
                    n_split += 1
                out.append(inst)
            blk.instructions = out
    return n_split


def _get_nc():
    key = ("nc", MM_MODE)
    if key not in _NC_CACHE:
        _NC_CACHE[key] = _build_nc()
    return _NC_CACHE[key]


def _make_in_maps(x_querry, K, A, p):
    x = np.ascontiguousarray(
        np.asarray(x_querry, dtype=np.float32).reshape(B * Q, D))
    Kf = np.ascontiguousarray(np.asarray(K, dtype=np.float32)[:F_END])
    Af = np.ascontiguousarray(np.asarray(A, dtype=np.float32)[:F_END])
    ps_flat = np.asarray(p, dtype=np.float32)[:F_END].reshape(F_END, NCOL)
    psf = np.ascontiguousarray(
        np.concatenate(
            [ps_flat[:, q * QCOL:(q + 1) * QCOL] for q in range(N_QUART)],
            axis=0))
    return [
        {"x": np.ascontiguousarray(x[i * ROWS:(i + 1) * ROWS]),
         "K": Kf, "A": Af, "ps": psf}
        for i in range(N_CORES)
    ]


def _assemble(results):
    out = np.empty((B * Q, NCOL), np.float32)
    for i in range(N_CORES):
        out[i * ROWS:(i + 1) * ROWS] = results[i]["out"]
    P_ = out.reshape(B, Q, E_P_LEN, P_FEAT)
    half = E_P_LEN // 2
    Ek = np.ascontiguousarray(P_[:, :, :half, :])
    Ev = np.ascontiguousarray(P_[:, :, half:, :])
    return Ek, Ev


def kernel(x_querry, l=None, x_block=None, K=None, A=None, p=None, **_kw):
    from concourse.bass_utils import run_bass_kernel_spmd

    nc = _get_nc()
    in_maps = _make_in_maps(x_querry, K, A, p)
    res = run_bass_kernel_spmd(nc, in_maps, core_ids=list(range(N_CORES)))
    return _assemble(res.results)


def kernel_traced(x_querry, l=None, x_block=None, K=None, A=None, p=None, **_kw):
    """Like kernel(), but also returns the profiled HW exec time in ns."""
    from concourse.bass_utils import run_bass_kernel_spmd

    nc = _get_nc()
    in_maps = _make_in_maps(x_querry, K, A, p)
    res = run_bass_kernel_spmd(
        nc, in_maps, core_ids=list(range(N_CORES)), trace=True)
    return _assemble(res.results), res.exec_time_ns

